# revision 37
# baseline (speedup 1.0000x reference)
"""Trainium2 Bass kernel for causal MHA (nn_MHA_18743237280339).

Full-input contract: kernel(**inputs) takes the unsharded numpy inputs and
returns the full [2, 4096, 512] output.

Distribution (8 NeuronCores, SPMD single program):
  - tensor-parallel over (batch, head): core i handles batch b=i//4 and
    heads h0=2*(i%4), h0+1. Projections use host-sliced weight columns, so
    every core runs an identical program on different data.
  - attention is flash-style: scores stay in PSUM, softmax denominator
    comes free from a ones-augmented V column (M=65 PV matmul), no
    max-subtraction (logits are tiny at this problem's scale).
  - the ScalarE exp stream is the roofline (~123us busy/core); the PSUM
    pools are sized (scores 3x2 banks + pv 2 banks) and allocation-ordered
    so QK^T stays ~2 groups ahead of exp, with next-block projection and
    previous-block epilogue interleaved into the PE slack.
  - output projection is computed LOCALLY as partials (wo columns for this
    core's 128 head-dims; wo output is full 512 wide), staged to DRAM, and
    summed across each batch's 4-core group by 4 token-chunked
    ReduceScatter(add) collectives that write the output shards directly.
    No AllGather of attention outputs at all.

Host-side work is limited to slicing/transposing/casting inputs and
reassembling the output.
"""

import math

import numpy as np
import ml_dtypes

import concourse.bass as bass
import concourse.bacc as bacc
import concourse.tile as tile
from concourse import mybir
from concourse.bass_utils import run_bass_kernel_spmd

BF16 = mybir.dt.bfloat16
F32 = mybir.dt.float32

D, H, B, S, HD = 512, 8, 2, 4096, 64
P = 128
NKT = D // P  # 4 contraction tiles of 128
NSB = S // 512  # 8 q-blocks of 512 rows
NCH = S // P  # 32 key chunks of 128
# ReduceScatter chunks, in q-blocks: front-loaded so the exposed tail
# collective (after the last block) is small.
RS_BLOCKS = [3, 3, 1, 1]
NRS = len(RS_BLOCKS)
RS_FIRST = [sum(RS_BLOCKS[:c]) for c in range(NRS)]  # first q-block of chunk

_CACHE: dict = {}


def _build_nc(body_reps=1, do_collective=True):
    nc = bacc.Bacc("TRN2", target_bir_lowering=False, debug=False, num_devices=8)

    xT_d = nc.declare_dram_parameter("xT", [D, S], BF16, isOutput=False)
    wq_d = nc.declare_dram_parameter("wqT", [D, P], BF16, isOutput=False)
    wk_d = nc.declare_dram_parameter("wkT", [D, P], BF16, isOutput=False)
    wv_d = nc.declare_dram_parameter("wvT", [D, P], BF16, isOutput=False)
    wo_d = nc.declare_dram_parameter("woT", [P, D], BF16, isOutput=False)
    bq_d = nc.declare_dram_parameter("bq", [P, 1], F32, isOutput=False)
    bk_d = nc.declare_dram_parameter("bk", [P, 1], F32, isOutput=False)
    wob_d = nc.declare_dram_parameter("wob", [P, NKT], F32, isOutput=False)
    mask_d = nc.declare_dram_parameter("masks", [4, P, 512], BF16, isOutput=False)
    out_d = [
        nc.declare_dram_parameter(f"outT{c}", [P, 512 * nb], BF16, isOutput=True)
        for c, nb in enumerate(RS_BLOCKS)
    ]

    with tile.TileContext(nc) as tc:
        for r in range(body_reps):
            _build_body(
                tc, xT_d, wq_d, wk_d, wv_d, wo_d, bq_d, bk_d, wob_d, mask_d, out_d,
                tag=f"r{r}", do_collective=do_collective,
            )

    nc.compile()
    return nc


def _build_body(
    tc, xT_d, wq_d, wk_d, wv_d, wo_d, bq_d, bk_d, wob_d, mask_d, out_d, tag="",
    do_collective=True,
):
    nc = tc.nc
    Exp = mybir.ActivationFunctionType.Exp

    with (
        tc.tile_pool(name=f"const{tag}", bufs=1) as const,
        tc.tile_pool(name=f"kqv{tag}", bufs=1) as kqv,
        tc.tile_pool(name=f"dram{tag}", bufs=1, space="DRAM") as dram,
        tc.tile_pool(name=f"xp{tag}", bufs=3) as xp,
        tc.tile_pool(name=f"sc{tag}", bufs=3, space="PSUM") as scp,  # 3x2 banks
        tc.tile_pool(name=f"pv{tag}", bufs=2, space="PSUM") as pvp,  # 2x1 banks
        tc.tile_pool(name=f"pt{tag}", bufs=4) as ptp,
        tc.tile_pool(name=f"rc{tag}", bufs=2) as rcp,
        tc.tile_pool(name=f"rbs{tag}", bufs=2) as rbsp,
        tc.tile_pool(name=f"st{tag}", bufs=2) as stp,
        tc.tile_pool(name=f"stt{tag}", bufs=2) as sttp,
        tc.tile_pool(name=f"stg{tag}", bufs=2) as stgp,
    ):
        # ---- constants (emission order = DMA queue order: the first
        # q-block's critical path needs wk/wq/masks before anything else;
        # the x-tile DMA for block 0 is emitted even earlier, below) ----
        wk_sb = const.tile([P, NKT, P], BF16, name=f"wk{tag}")
        wq_sb = const.tile([P, NKT, P], BF16, name=f"wq{tag}")
        mask_sb = const.tile([P, 4, 512], BF16, name=f"mask{tag}")
        wv_sb = const.tile([P, NKT, P], BF16, name=f"wv{tag}")
        wo_sb = const.tile([P, NKT, P], BF16, name=f"wo{tag}")
        bq_sb = const.tile([P, 1], F32, name=f"bq{tag}")
        bk_sb = const.tile([P, 1], F32, name=f"bk{tag}")
        wob_sb = const.tile([P, NKT], F32, name=f"wob{tag}")
        ones_bf = const.tile([P, HD], BF16, name=f"ones{tag}")

        def load_consts_first():
            # scalar-queue DMAs run in parallel with the sync-queue x-tile
            # loads; ScalarE is idle during startup anyway.
            nc.scalar.dma_start(
                wk_sb[:], wk_d[:, :].rearrange("(c p) m -> p c m", p=P)
            )
            nc.scalar.dma_start(
                wq_sb[:], wq_d[:, :].rearrange("(c p) m -> p c m", p=P)
            )

        def load_consts_early():
            nc.scalar.dma_start(bk_sb[:], bk_d[:, :])
            nc.scalar.dma_start(bq_sb[:], bq_d[:, :])
            for c in range(4):
                nc.scalar.dma_start(mask_sb[:, c, :], mask_d[c, :, :])

        def load_consts_late():
            nc.sync.dma_start(
                wv_sb[:], wv_d[:, :].rearrange("(c p) m -> p c m", p=P)
            )
            nc.sync.dma_start(
                wo_sb[:], wo_d[:, :].rearrange("p (c m) -> p c m", m=P)
            )
            nc.sync.dma_start(wob_sb[:], wob_d[:, :])
            nc.vector.memset(ones_bf[:], 1.0)

        # ---- persistent per-core tensors ----
        KT = kqv.tile([P, S], BF16, name=f"KT{tag}")  # 2 heads stacked (64+64)
        QT = kqv.tile([P, S], BF16, name=f"QT{tag}")
        V0 = kqv.tile([P, NCH, HD + 1], BF16, name=f"V0{tag}")
        V1 = kqv.tile([P, NCH, HD + 1], BF16, name=f"V1{tag}")
        nc.vector.memset(V0[:, :, HD : HD + 1], 1.0)
        nc.vector.memset(V1[:, :, HD : HD + 1], 1.0)

        partial = [
            dram.tile([D, 512 * nb], BF16, name=f"prt{c}{tag}")
            for c, nb in enumerate(RS_BLOCKS)
        ]
        # walrus forbids collectives writing IO tensors: RS lands in an
        # internal DRAM tile, then a DRAM->DRAM DMA copies to the output.
        rsout = [
            dram.tile([P, 512 * nb], BF16, name=f"rso{c}{tag}")
            for c, nb in enumerate(RS_BLOCKS)
        ]

        def proj_dma(j, xt_t, split=False):
            # x-tile DMA for q-block j, split per contraction chunk so the
            # first K-proj matmul can start after 1/4 of the transfer; the
            # startup block spreads chunks across both DMA queues.
            sl = slice(512 * j, 512 * (j + 1))
            xt = xp.tile([P, NKT, 512], BF16, tag="xt", name=f"xt{tag}_{j}")
            xt_t[j] = xt
            for kt in range(NKT):
                eng = nc.scalar if split and kt % 2 else nc.sync
                eng.dma_start(xt[:, kt, :], xT_d[P * kt : P * (kt + 1), sl])

        def proj_kq(j, xt_t):
            sl = slice(512 * j, 512 * (j + 1))
            xt = xt_t[j]
            pkq = scp.tile([P, 1024], F32, tag="sc", name=f"pkq{tag}_{j}")
            for kt in range(NKT):
                nc.tensor.matmul(
                    pkq[:, 0:512],
                    lhsT=wk_sb[:, kt, :],
                    rhs=xt[:, kt, :],
                    start=(kt == 0),
                    stop=(kt == NKT - 1),
                )
            for kt in range(NKT):
                nc.tensor.matmul(
                    pkq[:, 512:1024],
                    lhsT=wq_sb[:, kt, :],
                    rhs=xt[:, kt, :],
                    start=(kt == 0),
                    stop=(kt == NKT - 1),
                )
            nc.vector.tensor_scalar_add(KT[:, sl], pkq[:, 0:512], bk_sb[:])
            nc.vector.tensor_scalar_add(QT[:, sl], pkq[:, 512:1024], bq_sb[:])

        def proj_v(j, xt_t):
            # V projection: out[token, vdim(128)] per 128-token segment.
            xt = xt_t[j]
            pvv = scp.tile([P, 1024], F32, tag="sc", name=f"pvv{tag}_{j}")
            for t in range(4):
                for kt in range(NKT):
                    nc.tensor.matmul(
                        pvv[:, P * t : P * (t + 1)],
                        lhsT=xt[:, kt, P * t : P * (t + 1)],
                        rhs=wv_sb[:, kt, :],
                        start=(kt == 0),
                        stop=(kt == NKT - 1),
                    )
            for t in range(4):
                ch = 4 * j + t
                nc.vector.tensor_copy(V0[:, ch, 0:HD], pvv[:, P * t : P * t + HD])
                nc.vector.tensor_copy(
                    V1[:, ch, 0:HD], pvv[:, P * t + HD : P * (t + 1)]
                )

        def attn_qk(j, g):
            # QK^T + exp + mask for 2 key-chunks (2g, 2g+1) of q-block j.
            # For a diagonal chunk at offset m = kc-4j, the first 128*m query
            # columns are fully masked: QK/exp/PV all skip that prefix (the
            # stale pt prefix is never read).
            sp = [
                scp.tile([P, 1024], F32, tag="sc", name=f"sp{tag}_{p}_{j}_{g}")
                for p in range(2)
            ]
            lo = [max(0, P * (2 * g + t - 4 * j)) for t in range(2)]
            for t in range(2):
                kc = 2 * g + t
                for p in range(2):
                    base = HD * p
                    nc.tensor.matmul(
                        sp[p][:, 512 * t + lo[t] : 512 * (t + 1)],
                        lhsT=KT[base : base + HD, P * kc : P * (kc + 1)],
                        rhs=QT[base : base + HD, 512 * j + lo[t] : 512 * (j + 1)],
                        start=True,
                        stop=True,
                    )
            pt_ = [
                ptp.tile([P, 1024], BF16, tag="pt", name=f"pt{tag}_{p}_{j}_{g}")
                for p in range(2)
            ]
            for p in range(2):
                if lo[0] == 0 and lo[1] == 0:
                    nc.scalar.activation(pt_[p][:], sp[p][:], Exp)
                else:
                    for t in range(2):
                        nc.scalar.activation(
                            pt_[p][:, 512 * t + lo[t] : 512 * (t + 1)],
                            sp[p][:, 512 * t + lo[t] : 512 * (t + 1)],
                            Exp,
                        )
            for t in range(2):
                kc = 2 * g + t
                if kc >= 4 * j:
                    m = kc - 4 * j
                    for p in range(2):
                        nc.vector.tensor_mul(
                            pt_[p][:, 512 * t + lo[t] : 512 * (t + 1)],
                            pt_[p][:, 512 * t + lo[t] : 512 * (t + 1)],
                            mask_sb[:, m, lo[t] : 512],
                        )
            return pt_

        def attn_pv(j, g, pt_, pv):
            nch = 4 * (j + 1)
            for t in range(2):
                kc = 2 * g + t
                lo = max(0, P * (kc - 4 * j))
                for p in range(2):
                    Vp = V0 if p == 0 else V1
                    nc.tensor.matmul(
                        pv[p][0 : HD + 1, lo:512],
                        lhsT=Vp[:, kc, :],
                        rhs=pt_[p][:, 512 * t + lo : 512 * (t + 1)],
                        start=(kc == 0),
                        stop=(kc == nch - 1),
                    )

        st_t = {}

        def norm(j, pv):
            # softmax normalization: denominator reciprocal, broadcast over
            # the 64 head dims via a K=1 matmul, then scale; head1's half is
            # DMA-stacked under head0 so wo sees one [128, 512] rhs.
            rc = rcp.tile([P, 1024], BF16, tag="rc", name=f"rc{tag}_{j}")
            with nc.allow_low_precision(reason="bf16 softmax recip; 2e-2 gate"):
                nc.vector.reciprocal(rc[HD : HD + 1, 0:512], pv[0][HD : HD + 1, :])
                nc.vector.reciprocal(
                    rc[HD : HD + 1, 512:1024], pv[1][HD : HD + 1, :]
                )
            rb = scp.tile([P, 1024], F32, tag="sc", name=f"rb{tag}_{j}")
            for p in range(2):
                nc.tensor.matmul(
                    rb[0:HD, 512 * p : 512 * (p + 1)],
                    lhsT=ones_bf[HD : HD + 1, 0:HD],
                    rhs=rc[HD : HD + 1, 512 * p : 512 * (p + 1)],
                    start=True,
                    stop=True,
                )
            rbs = rbsp.tile([HD, 1024], BF16, tag="rbs", name=f"rbs{tag}_{j}")
            nc.vector.tensor_copy(rbs[:], rb[0:HD, :])
            st = stp.tile([P, 512], BF16, tag="st", name=f"st{tag}_{j}")
            st_t[j] = st
            stt = sttp.tile([HD, 512], BF16, tag="stt", name=f"stt{tag}_{j}")
            nc.vector.tensor_mul(st[0:HD, :], pv[0][0:HD, :], rbs[:, 0:512])
            nc.vector.tensor_mul(stt[:], pv[1][0:HD, :], rbs[:, 512:1024])
            nc.sync.dma_start(st[HD:P, :], stt[:])  # stack head1 under head0

        def wo_partial(j):
            # local partial wo: out[128-outs-block, 512 tokens] x4 blocks,
            # K=128 local head dims; bias added during the PSUM drain.
            st = st_t.pop(j)
            c = next(i for i in range(NRS - 1, -1, -1) if RS_FIRST[i] <= j)
            half = j - RS_FIRST[c]
            for h2 in range(2):
                wop = scp.tile([P, 1024], F32, tag="sc", name=f"wop{tag}_{j}_{h2}")
                stg = stgp.tile([P, 2, 512], BF16, tag="stg", name=f"stg{tag}_{j}_{h2}")
                for ob in range(2):
                    blk = 2 * h2 + ob
                    nc.tensor.matmul(
                        wop[:, 512 * ob : 512 * (ob + 1)],
                        lhsT=wo_sb[:, blk, :],
                        rhs=st[:],
                        start=True,
                        stop=True,
                    )
                    nc.vector.tensor_scalar_add(
                        stg[:, ob, :],
                        wop[:, 512 * ob : 512 * (ob + 1)],
                        wob_sb[:, blk : blk + 1],
                    )
                nc.sync.dma_start(
                    partial[c][
                        256 * h2 : 256 * (h2 + 1), 512 * half : 512 * (half + 1)
                    ].rearrange("(b p) t -> p b t", p=P),
                    stg[:],
                )

        def reduce_scatter(c):
            if do_collective:
                nc.gpsimd.collective_compute(
                    "ReduceScatter",
                    mybir.AluOpType.add,
                    replica_groups=[[0, 1, 2, 3], [4, 5, 6, 7]],
                    ins=[partial[c][:].opt()],
                    outs=[rsout[c][:].opt()],
                )
                # out-copy: gpsimd queue mid-kernel (orders behind its RS,
                # blocks nothing); sync queue for the last chunk (empty then)
                eng = nc.sync if c == NRS - 1 else nc.gpsimd
                eng.dma_start(out_d[c][:], rsout[c][:])
            else:
                nc.sync.dma_start(out_d[c][:], partial[c][0:P, :])

        # ---- schedule ----
        # Injection points spread sub-microsecond PE pieces across the
        # exp-paced groups so ScalarE never starves: after g0 the previous
        # block's normalization, after g1 its wo partials (+due RS), after
        # g2/g3 the next block's K/Q and V projections.
        pv_t = {}

        def make_pv(j):
            pv_t[j] = [
                pvp.tile([P, 512], F32, tag="pv", name=f"pvt{tag}_{p}_{j}")
                for p in range(2)
            ]

        xt_t = {}
        load_consts_first()
        proj_dma(0, xt_t)
        load_consts_early()
        proj_kq(0, xt_t)
        load_consts_late()
        proj_v(0, xt_t)
        make_pv(0)
        pending = []  # deferred PV: flushed one group behind QK
        for j in range(NSB):
            ng = 2 * (j + 1)
            if j + 1 < NSB:
                proj_dma(j + 1, xt_t)
            pieces = []
            if j > 0:
                pieces.append(lambda jj=j: norm(jj - 1, pv_t.pop(jj - 1)))

                def wo_rs(jj=j):
                    wo_partial(jj - 1)
                    for c in range(NRS):
                        if RS_FIRST[c] + RS_BLOCKS[c] == jj:
                            reduce_scatter(c)

                pieces.append(wo_rs)
            if j + 1 < NSB:
                pieces.append(lambda jj=j: proj_kq(jj + 1, xt_t))
                pieces.append(lambda jj=j: (proj_v(jj + 1, xt_t), make_pv(jj + 1)))
            for g in range(ng):
                pt_ = attn_qk(j, g)
                if pending:
                    attn_pv(*pending.pop())
                pending.append((j, g, pt_, pv_t[j]))
                if pieces and (g >= 1 or ng == 2):
                    pieces.pop(0)()
            while pieces:
                pieces.pop(0)()
        attn_pv(*pending.pop())
        norm(NSB - 1, pv_t.pop(NSB - 1))
        wo_partial(NSB - 1)
        reduce_scatter(NRS - 1)


def _get_nc():
    if "nc" not in _CACHE:
        _CACHE["nc"] = _build_nc()
    return _CACHE["nc"]


def _prepare_in_maps(x, wq_w, wq_b, wk_w, wk_b, wv_w, wv_b, wo_w, wo_b):
    bf16 = ml_dtypes.bfloat16
    f32 = np.float32
    x = np.asarray(x, f32)
    wq_w = np.asarray(wq_w, f32)
    wq_b = np.asarray(wq_b, f32)
    wk_w = np.asarray(wk_w, f32)
    wk_b = np.asarray(wk_b, f32)
    wv_w = np.asarray(wv_w, f32)
    wv_b = np.asarray(wv_b, f32)
    wo_w = np.asarray(wo_w, f32)
    wo_b = np.asarray(wo_b, f32)

    scale = f32(1.0 / math.sqrt(D))

    qi = np.arange(512)[None, :]
    ki = np.arange(P)[:, None]
    masks = np.stack(
        [(ki + 128 * c <= qi).astype(f32) for c in range(4)], axis=0
    )  # [4,128,512]
    masks_bf = np.ascontiguousarray(masks.astype(bf16))

    xT = [np.ascontiguousarray(x[b].T).astype(bf16) for b in range(B)]

    in_maps = []
    for i in range(8):
        b = i // 4
        r = i % 4
        h0 = 2 * r
        hs = slice(64 * h0, 64 * h0 + 128)
        # per-core wo bias: fold wv_b through this core's wo columns; the
        # full wo_b rides on group-rank 0 only (summed once by the RS).
        wob_core = wo_w[:, hs] @ wv_b[hs]
        if r == 0:
            wob_core = wob_core + wo_b
        in_maps.append(
            {
                "xT": xT[b],
                "wqT": np.ascontiguousarray((wq_w[hs, :] * scale).T).astype(bf16),
                "wkT": np.ascontiguousarray(wk_w[hs, :].T).astype(bf16),
                "wvT": np.ascontiguousarray(wv_w[hs, :].T).astype(bf16),
                "woT": np.ascontiguousarray(wo_w[:, hs].T).astype(bf16),
                "bq": np.ascontiguousarray((wq_b[hs] * scale).reshape(P, 1)),
                "bk": np.ascontiguousarray(wk_b[hs].reshape(P, 1)),
                "wob": np.ascontiguousarray(wob_core.reshape(NKT, P).T),
                "masks": masks_bf,
            }
        )
    return in_maps


def kernel(
    x, wq_w, wq_b, wk_w, wk_b, wv_w, wv_b, wo_w, wo_b, trace=False, **run_kwargs
):
    in_maps = _prepare_in_maps(x, wq_w, wq_b, wk_w, wk_b, wv_w, wv_b, wo_w, wo_b)
    res = run_bass_kernel_spmd(
        _get_nc(), in_maps, core_ids=list(range(8)), trace=trace, **run_kwargs
    )
    _CACHE["last_result"] = res
    out = np.zeros((B, S, D), np.float32)
    for i in range(8):
        b, r = i // 4, i % 4
        for c in range(NRS):
            oT = res.results[i][f"outT{c}"]  # [128, 512*nb]
            t0 = 512 * RS_FIRST[c]
            out[b, t0 : t0 + oT.shape[1], 128 * r : 128 * (r + 1)] = oT.T
    return out


# revision 43
# speedup vs baseline: 1.0189x; 1.0189x over previous
"""Trainium2 Bass kernel for causal MHA (nn_MHA_18743237280339).

Full-input contract: kernel(**inputs) takes the unsharded numpy inputs and
returns the full [2, 4096, 512] output.

Distribution (8 NeuronCores, SPMD single program):
  - tensor-parallel over (batch, head): core i handles batch b=i//4 and
    heads h0=2*(i%4), h0+1. Projections use host-sliced weight columns, so
    every core runs an identical program on different data.
  - attention is flash-style: scores stay in PSUM, softmax denominator
    comes free from a ones-augmented V column (M=65 PV matmul), no
    max-subtraction (logits are tiny at this problem's scale).
  - the ScalarE exp stream is the roofline (~123us busy/core); the PSUM
    pools are sized (scores 3x2 banks + pv 2 banks) and allocation-ordered
    so QK^T stays ~2 groups ahead of exp; PV matmuls are deferred up to 3
    groups behind QK (software pipelining against in-order engines), and
    next-block projection / previous-block epilogue are injected one
    sub-microsecond piece per group into the PE slack. Diagonal key-chunks
    skip their fully-masked 128*m query-column prefix in QK/exp/PV.
  - output projection is computed LOCALLY as partials (wo columns for this
    core's 128 head-dims; wo output is full 512 wide), staged to DRAM, and
    summed across each batch's 4-core group by 4 token-chunked
    ReduceScatter(add) collectives that write the output shards directly.
    No AllGather of attention outputs at all.

Host-side work is limited to slicing/transposing/casting inputs and
reassembling the output.
"""

import math

import numpy as np
import ml_dtypes

import concourse.bass as bass
import concourse.bacc as bacc
import concourse.tile as tile
from concourse import mybir
from concourse.bass_utils import run_bass_kernel_spmd

BF16 = mybir.dt.bfloat16
F32 = mybir.dt.float32

D, H, B, S, HD = 512, 8, 2, 4096, 64
P = 128
NKT = D // P  # 4 contraction tiles of 128
NSB = S // 512  # 8 q-blocks of 512 rows
NCH = S // P  # 32 key chunks of 128
# ReduceScatter chunks, in q-blocks: front-loaded so the exposed tail
# collective (after the last block) is small.
RS_BLOCKS = [3, 3, 1, 1]
NRS = len(RS_BLOCKS)
RS_FIRST = [sum(RS_BLOCKS[:c]) for c in range(NRS)]  # first q-block of chunk

_CACHE: dict = {}


def _build_nc(body_reps=1, do_collective=True):
    nc = bacc.Bacc("TRN2", target_bir_lowering=False, debug=False, num_devices=8)

    xT_d = nc.declare_dram_parameter("xT", [D, S], BF16, isOutput=False)
    wq_d = nc.declare_dram_parameter("wqT", [D, P], BF16, isOutput=False)
    wk_d = nc.declare_dram_parameter("wkT", [D, P], BF16, isOutput=False)
    wv_d = nc.declare_dram_parameter("wvT", [D, P], BF16, isOutput=False)
    wo_d = nc.declare_dram_parameter("woT", [P, D], BF16, isOutput=False)
    bq_d = nc.declare_dram_parameter("bq", [P, 1], F32, isOutput=False)
    bk_d = nc.declare_dram_parameter("bk", [P, 1], F32, isOutput=False)
    wob_d = nc.declare_dram_parameter("wob", [P, NKT], F32, isOutput=False)
    mask_d = nc.declare_dram_parameter("masks", [4, P, 512], BF16, isOutput=False)
    out_d = [
        nc.declare_dram_parameter(f"outT{c}", [P, 512 * nb], BF16, isOutput=True)
        for c, nb in enumerate(RS_BLOCKS)
    ]

    with tile.TileContext(nc) as tc:
        for r in range(body_reps):
            _build_body(
                tc, xT_d, wq_d, wk_d, wv_d, wo_d, bq_d, bk_d, wob_d, mask_d, out_d,
                tag=f"r{r}", do_collective=do_collective,
            )

    nc.compile()
    return nc


def _build_body(
    tc, xT_d, wq_d, wk_d, wv_d, wo_d, bq_d, bk_d, wob_d, mask_d, out_d, tag="",
    do_collective=True,
):
    nc = tc.nc
    Exp = mybir.ActivationFunctionType.Exp

    with (
        tc.tile_pool(name=f"const{tag}", bufs=1) as const,
        tc.tile_pool(name=f"kqv{tag}", bufs=1) as kqv,
        tc.tile_pool(name=f"dram{tag}", bufs=1, space="DRAM") as dram,
        tc.tile_pool(name=f"xp{tag}", bufs=3) as xp,
        tc.tile_pool(name=f"sc{tag}", bufs=3, space="PSUM") as scp,  # 3x2 banks
        tc.tile_pool(name=f"pv{tag}", bufs=2, space="PSUM") as pvp,  # 2x1 banks
        tc.tile_pool(name=f"pt{tag}", bufs=8) as ptp,
        tc.tile_pool(name=f"rc{tag}", bufs=2) as rcp,
        tc.tile_pool(name=f"rbs{tag}", bufs=2) as rbsp,
        tc.tile_pool(name=f"st{tag}", bufs=2) as stp,
        tc.tile_pool(name=f"stt{tag}", bufs=2) as sttp,
        tc.tile_pool(name=f"stg{tag}", bufs=2) as stgp,
    ):
        # ---- constants (emission order = DMA queue order: the first
        # q-block's critical path needs wk/wq/masks before anything else;
        # the x-tile DMA for block 0 is emitted even earlier, below) ----
        wk_sb = const.tile([P, NKT, P], BF16, name=f"wk{tag}")
        wq_sb = const.tile([P, NKT, P], BF16, name=f"wq{tag}")
        mask_sb = const.tile([P, 4, 512], BF16, name=f"mask{tag}")
        wv_sb = const.tile([P, NKT, P], BF16, name=f"wv{tag}")
        wo_sb = const.tile([P, NKT, P], BF16, name=f"wo{tag}")
        bq_sb = const.tile([P, 1], F32, name=f"bq{tag}")
        bk_sb = const.tile([P, 1], F32, name=f"bk{tag}")
        wob_sb = const.tile([P, NKT], F32, name=f"wob{tag}")
        ones_bf = const.tile([P, HD], BF16, name=f"ones{tag}")

        def load_consts_first():
            # scalar-queue DMAs run in parallel with the sync-queue x-tile
            # loads; ScalarE is idle during startup anyway.
            nc.scalar.dma_start(
                wk_sb[:], wk_d[:, :].rearrange("(c p) m -> p c m", p=P)
            )
            nc.scalar.dma_start(
                wq_sb[:], wq_d[:, :].rearrange("(c p) m -> p c m", p=P)
            )

        def load_consts_early():
            nc.scalar.dma_start(bk_sb[:], bk_d[:, :])
            nc.scalar.dma_start(bq_sb[:], bq_d[:, :])
            for c in range(4):
                nc.scalar.dma_start(mask_sb[:, c, :], mask_d[c, :, :])

        def load_consts_late():
            nc.sync.dma_start(
                wv_sb[:], wv_d[:, :].rearrange("(c p) m -> p c m", p=P)
            )
            nc.sync.dma_start(
                wo_sb[:], wo_d[:, :].rearrange("p (c m) -> p c m", m=P)
            )
            nc.sync.dma_start(wob_sb[:], wob_d[:, :])
            nc.vector.memset(ones_bf[:], 1.0)

        # ---- persistent per-core tensors ----
        KT = kqv.tile([P, S], BF16, name=f"KT{tag}")  # 2 heads stacked (64+64)
        QT = kqv.tile([P, S], BF16, name=f"QT{tag}")
        V0 = kqv.tile([P, NCH, HD + 1], BF16, name=f"V0{tag}")
        V1 = kqv.tile([P, NCH, HD + 1], BF16, name=f"V1{tag}")
        nc.vector.memset(V0[:, :, HD : HD + 1], 1.0)
        nc.vector.memset(V1[:, :, HD : HD + 1], 1.0)

        partial = [
            dram.tile([D, 512 * nb], BF16, name=f"prt{c}{tag}")
            for c, nb in enumerate(RS_BLOCKS)
        ]
        # walrus forbids collectives writing IO tensors: RS lands in an
        # internal DRAM tile, then a DRAM->DRAM DMA copies to the output.
        rsout = [
            dram.tile([P, 512 * nb], BF16, name=f"rso{c}{tag}")
            for c, nb in enumerate(RS_BLOCKS)
        ]

        def proj_dma(j, xt_t, split=False):
            # x-tile DMA for q-block j, split per contraction chunk so the
            # first K-proj matmul can start after 1/4 of the transfer; the
            # startup block spreads chunks across both DMA queues.
            sl = slice(512 * j, 512 * (j + 1))
            xt = xp.tile([P, NKT, 512], BF16, tag="xt", name=f"xt{tag}_{j}")
            xt_t[j] = xt
            for kt in range(NKT):
                eng = nc.scalar if split and kt % 2 else nc.sync
                eng.dma_start(xt[:, kt, :], xT_d[P * kt : P * (kt + 1), sl])

        def proj_kq(j, xt_t):
            sl = slice(512 * j, 512 * (j + 1))
            xt = xt_t[j]
            pkq = scp.tile([P, 1024], F32, tag="sc", name=f"pkq{tag}_{j}")
            for kt in range(NKT):
                nc.tensor.matmul(
                    pkq[:, 0:512],
                    lhsT=wk_sb[:, kt, :],
                    rhs=xt[:, kt, :],
                    start=(kt == 0),
                    stop=(kt == NKT - 1),
                )
            for kt in range(NKT):
                nc.tensor.matmul(
                    pkq[:, 512:1024],
                    lhsT=wq_sb[:, kt, :],
                    rhs=xt[:, kt, :],
                    start=(kt == 0),
                    stop=(kt == NKT - 1),
                )
            nc.vector.tensor_scalar_add(KT[:, sl], pkq[:, 0:512], bk_sb[:])
            nc.vector.tensor_scalar_add(QT[:, sl], pkq[:, 512:1024], bq_sb[:])

        def proj_v(j, xt_t):
            # V projection: out[token, vdim(128)] per 128-token segment.
            xt = xt_t[j]
            pvv = scp.tile([P, 1024], F32, tag="sc", name=f"pvv{tag}_{j}")
            for t in range(4):
                for kt in range(NKT):
                    nc.tensor.matmul(
                        pvv[:, P * t : P * (t + 1)],
                        lhsT=xt[:, kt, P * t : P * (t + 1)],
                        rhs=wv_sb[:, kt, :],
                        start=(kt == 0),
                        stop=(kt == NKT - 1),
                    )
            for t in range(4):
                ch = 4 * j + t
                nc.vector.tensor_copy(V0[:, ch, 0:HD], pvv[:, P * t : P * t + HD])
                nc.vector.tensor_copy(
                    V1[:, ch, 0:HD], pvv[:, P * t + HD : P * (t + 1)]
                )

        def attn_qk(j, g):
            # QK^T + exp + mask for 2 key-chunks (2g, 2g+1) of q-block j.
            # For a diagonal chunk at offset m = kc-4j, the first 128*m query
            # columns are fully masked: QK/exp/PV all skip that prefix (the
            # stale pt prefix is never read).
            sp = [
                scp.tile([P, 1024], F32, tag="sc", name=f"sp{tag}_{p}_{j}_{g}")
                for p in range(2)
            ]
            lo = [max(0, P * (2 * g + t - 4 * j)) for t in range(2)]
            for t in range(2):
                kc = 2 * g + t
                for p in range(2):
                    base = HD * p
                    nc.tensor.matmul(
                        sp[p][:, 512 * t + lo[t] : 512 * (t + 1)],
                        lhsT=KT[base : base + HD, P * kc : P * (kc + 1)],
                        rhs=QT[base : base + HD, 512 * j + lo[t] : 512 * (j + 1)],
                        start=True,
                        stop=True,
                    )
            pt_ = [
                ptp.tile([P, 1024], BF16, tag="pt", name=f"pt{tag}_{p}_{j}_{g}")
                for p in range(2)
            ]
            for p in range(2):
                if lo[0] == 0 and lo[1] == 0:
                    nc.scalar.activation(pt_[p][:], sp[p][:], Exp)
                else:
                    for t in range(2):
                        nc.scalar.activation(
                            pt_[p][:, 512 * t + lo[t] : 512 * (t + 1)],
                            sp[p][:, 512 * t + lo[t] : 512 * (t + 1)],
                            Exp,
                        )
            for t in range(2):
                kc = 2 * g + t
                if kc >= 4 * j:
                    m = kc - 4 * j
                    for p in range(2):
                        nc.vector.tensor_mul(
                            pt_[p][:, 512 * t + lo[t] : 512 * (t + 1)],
                            pt_[p][:, 512 * t + lo[t] : 512 * (t + 1)],
                            mask_sb[:, m, lo[t] : 512],
                        )
            return pt_

        def attn_pv(j, g, pt_, pv):
            nch = 4 * (j + 1)
            for t in range(2):
                kc = 2 * g + t
                lo = max(0, P * (kc - 4 * j))
                for p in range(2):
                    Vp = V0 if p == 0 else V1
                    nc.tensor.matmul(
                        pv[p][0 : HD + 1, lo:512],
                        lhsT=Vp[:, kc, :],
                        rhs=pt_[p][:, 512 * t + lo : 512 * (t + 1)],
                        start=(kc == 0),
                        stop=(kc == nch - 1),
                    )

        st_t = {}

        def norm(j, pv):
            # softmax normalization: denominator reciprocal, broadcast over
            # the 64 head dims via a K=1 matmul, then scale; head1's half is
            # DMA-stacked under head0 so wo sees one [128, 512] rhs.
            rc = rcp.tile([P, 1024], BF16, tag="rc", name=f"rc{tag}_{j}")
            with nc.allow_low_precision(reason="bf16 softmax recip; 2e-2 gate"):
                nc.vector.reciprocal(rc[HD : HD + 1, 0:512], pv[0][HD : HD + 1, :])
                nc.vector.reciprocal(
                    rc[HD : HD + 1, 512:1024], pv[1][HD : HD + 1, :]
                )
            rb = scp.tile([P, 1024], F32, tag="sc", name=f"rb{tag}_{j}")
            for p in range(2):
                nc.tensor.matmul(
                    rb[0:HD, 512 * p : 512 * (p + 1)],
                    lhsT=ones_bf[HD : HD + 1, 0:HD],
                    rhs=rc[HD : HD + 1, 512 * p : 512 * (p + 1)],
                    start=True,
                    stop=True,
                )
            rbs = rbsp.tile([HD, 1024], BF16, tag="rbs", name=f"rbs{tag}_{j}")
            nc.vector.tensor_copy(rbs[:], rb[0:HD, :])
            st = stp.tile([P, 512], BF16, tag="st", name=f"st{tag}_{j}")
            st_t[j] = st
            stt = sttp.tile([HD, 512], BF16, tag="stt", name=f"stt{tag}_{j}")
            nc.vector.tensor_mul(st[0:HD, :], pv[0][0:HD, :], rbs[:, 0:512])
            nc.vector.tensor_mul(stt[:], pv[1][0:HD, :], rbs[:, 512:1024])
            # stack head1 under head0; the last block uses the idle ACT
            # queue to dodge sync-queue latency in the exposed tail
            eng = nc.scalar if j == NSB - 1 else nc.sync
            eng.dma_start(st[HD:P, :], stt[:])

        def wo_partial(j):
            # local partial wo: out[128-outs-block, 512 tokens] x4 blocks,
            # K=128 local head dims; bias added during the PSUM drain.
            st = st_t.pop(j)
            c = next(i for i in range(NRS - 1, -1, -1) if RS_FIRST[i] <= j)
            half = j - RS_FIRST[c]
            for h2 in range(2):
                wop = scp.tile([P, 1024], F32, tag="sc", name=f"wop{tag}_{j}_{h2}")
                stg = stgp.tile([P, 2, 512], BF16, tag="stg", name=f"stg{tag}_{j}_{h2}")
                for ob in range(2):
                    blk = 2 * h2 + ob
                    nc.tensor.matmul(
                        wop[:, 512 * ob : 512 * (ob + 1)],
                        lhsT=wo_sb[:, blk, :],
                        rhs=st[:],
                        start=True,
                        stop=True,
                    )
                    nc.vector.tensor_scalar_add(
                        stg[:, ob, :],
                        wop[:, 512 * ob : 512 * (ob + 1)],
                        wob_sb[:, blk : blk + 1],
                    )
                eng2 = nc.scalar if j == NSB - 1 and h2 == 1 else nc.sync
                eng2.dma_start(
                    partial[c][
                        256 * h2 : 256 * (h2 + 1), 512 * half : 512 * (half + 1)
                    ].rearrange("(b p) t -> p b t", p=P),
                    stg[:],
                )

        def reduce_scatter(c):
            if do_collective:
                nc.gpsimd.collective_compute(
                    "ReduceScatter",
                    mybir.AluOpType.add,
                    replica_groups=[[0, 1, 2, 3], [4, 5, 6, 7]],
                    ins=[partial[c][:].opt()],
                    outs=[rsout[c][:].opt()],
                )
                # out-copy: gpsimd queue mid-kernel (orders behind its RS,
                # blocks nothing); the last chunk splits across the two idle
                # HWDGE queues to halve the exposed tail copy
                if c == NRS - 1:
                    nc.sync.dma_start(out_d[c][0 : P // 2, :], rsout[c][0 : P // 2, :])
                    nc.scalar.dma_start(out_d[c][P // 2 : P, :], rsout[c][P // 2 : P, :])
                else:
                    nc.gpsimd.dma_start(out_d[c][:], rsout[c][:])
            else:
                nc.sync.dma_start(out_d[c][:], partial[c][0:P, :])

        # ---- schedule ----
        # Injection points spread sub-microsecond PE pieces across the
        # exp-paced groups so ScalarE never starves: after g0 the previous
        # block's normalization, after g1 its wo partials (+due RS), after
        # g2/g3 the next block's K/Q and V projections.
        pv_t = {}

        def make_pv(j):
            pv_t[j] = [
                pvp.tile([P, 512], F32, tag="pv", name=f"pvt{tag}_{p}_{j}")
                for p in range(2)
            ]

        xt_t = {}
        load_consts_first()
        proj_dma(0, xt_t)
        load_consts_early()
        proj_kq(0, xt_t)
        load_consts_late()
        proj_v(0, xt_t)
        make_pv(0)
        pending = []  # deferred PV: flushed one group behind QK
        for j in range(NSB):
            ng = 2 * (j + 1)
            if j + 1 < NSB:
                proj_dma(j + 1, xt_t)
            pieces = []
            if j > 0:

                def norm_piece(jj=j):
                    while pending and pending[0][0] == jj - 1:
                        attn_pv(*pending.pop(0))
                    norm(jj - 1, pv_t.pop(jj - 1))

                pieces.append(norm_piece)

                def wo_rs(jj=j):
                    wo_partial(jj - 1)
                    for c in range(NRS):
                        if RS_FIRST[c] + RS_BLOCKS[c] == jj:
                            reduce_scatter(c)

                pieces.append(wo_rs)
            if j + 1 < NSB:
                pieces.append(lambda jj=j: proj_kq(jj + 1, xt_t))
                pieces.append(lambda jj=j: (proj_v(jj + 1, xt_t), make_pv(jj + 1)))
            for g in range(ng):
                pt_ = attn_qk(j, g)
                if len(pending) >= 3:
                    attn_pv(*pending.pop(0))
                pending.append((j, g, pt_, pv_t[j]))
                if pieces and (g >= 1 or ng == 2):
                    pieces.pop(0)()
            while pieces:
                pieces.pop(0)()
        while pending:
            attn_pv(*pending.pop(0))
        norm(NSB - 1, pv_t.pop(NSB - 1))
        wo_partial(NSB - 1)
        reduce_scatter(NRS - 1)


def _get_nc():
    if "nc" not in _CACHE:
        _CACHE["nc"] = _build_nc()
    return _CACHE["nc"]


def _prepare_in_maps(x, wq_w, wq_b, wk_w, wk_b, wv_w, wv_b, wo_w, wo_b):
    bf16 = ml_dtypes.bfloat16
    f32 = np.float32
    x = np.asarray(x, f32)
    wq_w = np.asarray(wq_w, f32)
    wq_b = np.asarray(wq_b, f32)
    wk_w = np.asarray(wk_w, f32)
    wk_b = np.asarray(wk_b, f32)
    wv_w = np.asarray(wv_w, f32)
    wv_b = np.asarray(wv_b, f32)
    wo_w = np.asarray(wo_w, f32)
    wo_b = np.asarray(wo_b, f32)

    scale = f32(1.0 / math.sqrt(D))

    qi = np.arange(512)[None, :]
    ki = np.arange(P)[:, None]
    masks = np.stack(
        [(ki + 128 * c <= qi).astype(f32) for c in range(4)], axis=0
    )  # [4,128,512]
    masks_bf = np.ascontiguousarray(masks.astype(bf16))

    xT = [np.ascontiguousarray(x[b].T).astype(bf16) for b in range(B)]

    in_maps = []
    for i in range(8):
        b = i // 4
        r = i % 4
        h0 = 2 * r
        hs = slice(64 * h0, 64 * h0 + 128)
        # per-core wo bias: fold wv_b through this core's wo columns; the
        # full wo_b rides on group-rank 0 only (summed once by the RS).
        wob_core = wo_w[:, hs] @ wv_b[hs]
        if r == 0:
            wob_core = wob_core + wo_b
        in_maps.append(
            {
                "xT": xT[b],
                "wqT": np.ascontiguousarray((wq_w[hs, :] * scale).T).astype(bf16),
                "wkT": np.ascontiguousarray(wk_w[hs, :].T).astype(bf16),
                "wvT": np.ascontiguousarray(wv_w[hs, :].T).astype(bf16),
                "woT": np.ascontiguousarray(wo_w[:, hs].T).astype(bf16),
                "bq": np.ascontiguousarray((wq_b[hs] * scale).reshape(P, 1)),
                "bk": np.ascontiguousarray(wk_b[hs].reshape(P, 1)),
                "wob": np.ascontiguousarray(wob_core.reshape(NKT, P).T),
                "masks": masks_bf,
            }
        )
    return in_maps


def kernel(
    x, wq_w, wq_b, wk_w, wk_b, wv_w, wv_b, wo_w, wo_b, trace=False, **run_kwargs
):
    in_maps = _prepare_in_maps(x, wq_w, wq_b, wk_w, wk_b, wv_w, wv_b, wo_w, wo_b)
    res = run_bass_kernel_spmd(
        _get_nc(), in_maps, core_ids=list(range(8)), trace=trace, **run_kwargs
    )
    _CACHE["last_result"] = res
    out = np.zeros((B, S, D), np.float32)
    for i in range(8):
        b, r = i // 4, i % 4
        for c in range(NRS):
            oT = res.results[i][f"outT{c}"]  # [128, 512*nb]
            t0 = 512 * RS_FIRST[c]
            out[b, t0 : t0 + oT.shape[1], 128 * r : 128 * (r + 1)] = oT.T
    return out


# revision 50
# speedup vs baseline: 1.0738x; 1.0538x over previous
"""Trainium2 Bass kernel for causal MHA (nn_MHA_18743237280339).

Full-input contract: kernel(**inputs) takes the unsharded numpy inputs and
returns the full [2, 4096, 512] output.

Distribution (8 NeuronCores, SPMD single program):
  - tensor-parallel over (batch, head): core i handles batch b=i//4 and
    heads h0=2*(i%4), h0+1. Projections use host-sliced weight columns, so
    every core runs an identical program on different data.
  - attention is flash-style: scores stay in PSUM, softmax denominator
    comes free from a ones-augmented V column (M=65 PV matmul), no
    max-subtraction (logits are tiny at this problem's scale).
  - the ScalarE exp stream is the roofline (~123us busy/core); the PSUM
    pools are sized (scores 3x2 banks + pv 2 banks) and allocation-ordered
    so QK^T stays ~2 groups ahead of exp; PV matmuls are deferred up to 3
    groups behind QK (software pipelining against in-order engines), and
    next-block projection / previous-block epilogue are injected one
    sub-microsecond piece per group into the PE slack. Diagonal key-chunks
    skip their fully-masked 128*m query-column prefix in QK/exp/PV.
  - output projection is computed LOCALLY as partials (wo columns for this
    core's 128 head-dims; wo output is full 512 wide), staged to DRAM, and
    summed across each batch's 4-core group by 4 token-chunked
    ReduceScatter(add) collectives that write the output shards directly.
    No AllGather of attention outputs at all.

Host-side work is limited to slicing/transposing/casting inputs and
reassembling the output.
"""

import math

import numpy as np
import ml_dtypes

import concourse.bass as bass
import concourse.bacc as bacc
import concourse.tile as tile
from concourse import mybir
from concourse.bass_utils import run_bass_kernel_spmd

BF16 = mybir.dt.bfloat16
F32 = mybir.dt.float32

D, H, B, S, HD = 512, 8, 2, 4096, 64
P = 128
NKT = D // P  # 4 contraction tiles of 128
NSB = S // 512  # 8 q-blocks of 512 rows
NCH = S // P  # 32 key chunks of 128
# ReduceScatter chunks, in q-blocks: front-loaded so the exposed tail
# collective (after the last block) is small.
RS_BLOCKS = [3, 3, 1, 1]
NRS = len(RS_BLOCKS)
RS_FIRST = [sum(RS_BLOCKS[:c]) for c in range(NRS)]  # first q-block of chunk

_CACHE: dict = {}


def _build_nc(body_reps=1, do_collective=True):
    nc = bacc.Bacc("TRN2", target_bir_lowering=False, debug=False, num_devices=8)

    xT_d = nc.declare_dram_parameter("xT", [D, S], BF16, isOutput=False)
    wq_d = nc.declare_dram_parameter("wqT", [D, P], BF16, isOutput=False)
    wk_d = nc.declare_dram_parameter("wkT", [D, P], BF16, isOutput=False)
    wv_d = nc.declare_dram_parameter("wvT", [D, P], BF16, isOutput=False)
    wo_d = nc.declare_dram_parameter("woT", [P, D], BF16, isOutput=False)
    bq_d = nc.declare_dram_parameter("bq", [P, 1], F32, isOutput=False)
    bk_d = nc.declare_dram_parameter("bk", [P, 1], F32, isOutput=False)
    wob_d = nc.declare_dram_parameter("wob", [P, NKT], F32, isOutput=False)
    mask_d = nc.declare_dram_parameter("masks", [4, P, 512], BF16, isOutput=False)
    out_d = [
        nc.declare_dram_parameter(f"outT{c}", [P, 512 * nb], BF16, isOutput=True)
        for c, nb in enumerate(RS_BLOCKS)
    ]

    with tile.TileContext(nc) as tc:
        for r in range(body_reps):
            _build_body(
                tc, xT_d, wq_d, wk_d, wv_d, wo_d, bq_d, bk_d, wob_d, mask_d, out_d,
                tag=f"r{r}", do_collective=do_collective,
            )

    nc.compile()
    return nc


def _build_body(
    tc, xT_d, wq_d, wk_d, wv_d, wo_d, bq_d, bk_d, wob_d, mask_d, out_d, tag="",
    do_collective=True,
):
    nc = tc.nc
    Exp = mybir.ActivationFunctionType.Exp

    with (
        tc.tile_pool(name=f"const{tag}", bufs=1) as const,
        tc.tile_pool(name=f"kqv{tag}", bufs=1) as kqv,
        tc.tile_pool(name=f"dram{tag}", bufs=1, space="DRAM") as dram,
        tc.tile_pool(name=f"xp{tag}", bufs=3) as xp,
        tc.tile_pool(name=f"sc{tag}", bufs=3, space="PSUM") as scp,  # 3x2 banks
        tc.tile_pool(name=f"pv{tag}", bufs=2, space="PSUM") as pvp,  # 2x1 banks
        tc.tile_pool(name=f"pt{tag}", bufs=8) as ptp,
        tc.tile_pool(name=f"rc{tag}", bufs=2) as rcp,
        tc.tile_pool(name=f"rbs{tag}", bufs=2) as rbsp,
        tc.tile_pool(name=f"st{tag}", bufs=2) as stp,
        tc.tile_pool(name=f"stt{tag}", bufs=2) as sttp,
        tc.tile_pool(name=f"stg{tag}", bufs=2) as stgp,
    ):
        # ---- constants (emission order = DMA queue order: the first
        # q-block's critical path needs wk/wq/masks before anything else;
        # the x-tile DMA for block 0 is emitted even earlier, below) ----
        wk_sb = const.tile([P, NKT, P], BF16, name=f"wk{tag}")
        wq_sb = const.tile([P, NKT, P], BF16, name=f"wq{tag}")
        mask_sb = const.tile([P, 4, 512], BF16, name=f"mask{tag}")
        wv_sb = const.tile([P, NKT, P], BF16, name=f"wv{tag}")
        wo_sb = const.tile([P, NKT, P], BF16, name=f"wo{tag}")
        bq_sb = const.tile([P, 1], F32, name=f"bq{tag}")
        bk_sb = const.tile([P, 1], F32, name=f"bk{tag}")
        wob_sb = const.tile([P, NKT], F32, name=f"wob{tag}")
        ones_bf = const.tile([P, HD], BF16, name=f"ones{tag}")

        def load_consts_first():
            # scalar-queue DMAs run in parallel with the sync-queue x-tile
            # loads; ScalarE is idle during startup anyway.
            nc.scalar.dma_start(
                wk_sb[:], wk_d[:, :].rearrange("(c p) m -> p c m", p=P)
            )
            nc.scalar.dma_start(
                wq_sb[:], wq_d[:, :].rearrange("(c p) m -> p c m", p=P)
            )

        def load_consts_early():
            nc.scalar.dma_start(bk_sb[:], bk_d[:, :])
            nc.scalar.dma_start(bq_sb[:], bq_d[:, :])
            for c in range(4):
                nc.scalar.dma_start(mask_sb[:, c, :], mask_d[c, :, :])

        def load_consts_late():
            nc.sync.dma_start(
                wv_sb[:], wv_d[:, :].rearrange("(c p) m -> p c m", p=P)
            )
            nc.sync.dma_start(
                wo_sb[:], wo_d[:, :].rearrange("p (c m) -> p c m", m=P)
            )
            nc.sync.dma_start(wob_sb[:], wob_d[:, :])
            nc.vector.memset(ones_bf[:], 1.0)

        # ---- persistent per-core tensors ----
        KT = kqv.tile([P, S], BF16, name=f"KT{tag}")  # 2 heads stacked (64+64)
        QT = kqv.tile([P, S], BF16, name=f"QT{tag}")
        V0 = kqv.tile([P, NCH, HD + 1], BF16, name=f"V0{tag}")
        V1 = kqv.tile([P, NCH, HD + 1], BF16, name=f"V1{tag}")
        nc.vector.memset(V0[:, :, HD : HD + 1], 1.0)
        nc.vector.memset(V1[:, :, HD : HD + 1], 1.0)

        partial = [
            dram.tile([D, 512 * nb], BF16, name=f"prt{c}{tag}")
            for c, nb in enumerate(RS_BLOCKS)
        ]
        # walrus forbids collectives writing IO tensors: RS lands in an
        # internal DRAM tile, then a DRAM->DRAM DMA copies to the output.
        rsout = [
            dram.tile([P, 512 * nb], BF16, name=f"rso{c}{tag}")
            for c, nb in enumerate(RS_BLOCKS)
        ]

        def proj_dma(j, xt_t, split=False):
            # x-tile DMA for q-block j, split per contraction chunk so the
            # first K-proj matmul can start after 1/4 of the transfer; the
            # startup block spreads chunks across both DMA queues.
            sl = slice(512 * j, 512 * (j + 1))
            xt = xp.tile([P, NKT, 512], BF16, tag="xt", name=f"xt{tag}_{j}")
            xt_t[j] = xt
            for kt in range(NKT):
                eng = nc.scalar if split and kt % 2 else nc.sync
                eng.dma_start(xt[:, kt, :], xT_d[P * kt : P * (kt + 1), sl])

        def proj_kq(j, xt_t):
            sl = slice(512 * j, 512 * (j + 1))
            xt = xt_t[j]
            pkq = scp.tile([P, 1024], F32, tag="sc", name=f"pkq{tag}_{j}")
            for kt in range(NKT):
                nc.tensor.matmul(
                    pkq[:, 0:512],
                    lhsT=wk_sb[:, kt, :],
                    rhs=xt[:, kt, :],
                    start=(kt == 0),
                    stop=(kt == NKT - 1),
                )
            for kt in range(NKT):
                nc.tensor.matmul(
                    pkq[:, 512:1024],
                    lhsT=wq_sb[:, kt, :],
                    rhs=xt[:, kt, :],
                    start=(kt == 0),
                    stop=(kt == NKT - 1),
                )
            nc.vector.tensor_scalar_add(KT[:, sl], pkq[:, 0:512], bk_sb[:])
            nc.vector.tensor_scalar_add(QT[:, sl], pkq[:, 512:1024], bq_sb[:])

        def proj_v(j, xt_t):
            # V projection: out[token, vdim(128)] per 128-token segment.
            xt = xt_t[j]
            pvv = scp.tile([P, 1024], F32, tag="sc", name=f"pvv{tag}_{j}")
            for t in range(4):
                for kt in range(NKT):
                    nc.tensor.matmul(
                        pvv[:, P * t : P * (t + 1)],
                        lhsT=xt[:, kt, P * t : P * (t + 1)],
                        rhs=wv_sb[:, kt, :],
                        start=(kt == 0),
                        stop=(kt == NKT - 1),
                    )
            for t in range(4):
                ch = 4 * j + t
                nc.vector.tensor_copy(V0[:, ch, 0:HD], pvv[:, P * t : P * t + HD])
                nc.vector.tensor_copy(
                    V1[:, ch, 0:HD], pvv[:, P * t + HD : P * (t + 1)]
                )

        def attn_qk(j, g):
            # QK^T + exp + mask for 2 key-chunks (2g, 2g+1) of q-block j.
            # For a diagonal chunk at offset m = kc-4j, the first 128*m query
            # columns are fully masked: QK/exp/PV all skip that prefix (the
            # stale pt prefix is never read).
            sp = [
                scp.tile([P, 1024], F32, tag="sc", name=f"sp{tag}_{p}_{j}_{g}")
                for p in range(2)
            ]
            lo = [max(0, P * (2 * g + t - 4 * j)) for t in range(2)]
            for t in range(2):
                kc = 2 * g + t
                for p in range(2):
                    base = HD * p
                    nc.tensor.matmul(
                        sp[p][:, 512 * t + lo[t] : 512 * (t + 1)],
                        lhsT=KT[base : base + HD, P * kc : P * (kc + 1)],
                        rhs=QT[base : base + HD, 512 * j + lo[t] : 512 * (j + 1)],
                        start=True,
                        stop=True,
                    )
            pt_ = [
                ptp.tile([P, 1024], BF16, tag="pt", name=f"pt{tag}_{p}_{j}_{g}")
                for p in range(2)
            ]
            for p in range(2):
                if lo[0] == 0 and lo[1] == 0:
                    nc.scalar.activation(pt_[p][:], sp[p][:], Exp)
                else:
                    for t in range(2):
                        nc.scalar.activation(
                            pt_[p][:, 512 * t + lo[t] : 512 * (t + 1)],
                            sp[p][:, 512 * t + lo[t] : 512 * (t + 1)],
                            Exp,
                        )
            for t in range(2):
                kc = 2 * g + t
                if kc >= 4 * j:
                    m = kc - 4 * j
                    for p in range(2):
                        nc.vector.tensor_mul(
                            pt_[p][:, 512 * t + lo[t] : 512 * (t + 1)],
                            pt_[p][:, 512 * t + lo[t] : 512 * (t + 1)],
                            mask_sb[:, m, lo[t] : 512],
                        )
            return pt_

        def attn_pv(j, g, pt_, pv):
            nch = 4 * (j + 1)
            for t in range(2):
                kc = 2 * g + t
                lo = max(0, P * (kc - 4 * j))
                for p in range(2):
                    Vp = V0 if p == 0 else V1
                    nc.tensor.matmul(
                        pv[p][0 : HD + 1, lo:512],
                        lhsT=Vp[:, kc, :],
                        rhs=pt_[p][:, 512 * t + lo : 512 * (t + 1)],
                        start=(kc == 0),
                        stop=(kc == nch - 1),
                    )

        st_t = {}

        def norm(j, pv):
            # softmax normalization: denominator reciprocal, broadcast over
            # the 64 head dims via a K=1 matmul, then scale; head1's half is
            # DMA-stacked under head0 so wo sees one [128, 512] rhs.
            rc = rcp.tile([P, 1024], BF16, tag="rc", name=f"rc{tag}_{j}")
            with nc.allow_low_precision(reason="bf16 softmax recip; 2e-2 gate"):
                nc.vector.reciprocal(rc[HD : HD + 1, 0:512], pv[0][HD : HD + 1, :])
                nc.vector.reciprocal(
                    rc[HD : HD + 1, 512:1024], pv[1][HD : HD + 1, :]
                )
            rb = scp.tile([P, 1024], F32, tag="sc", name=f"rb{tag}_{j}")
            for p in range(2):
                nc.tensor.matmul(
                    rb[0:HD, 512 * p : 512 * (p + 1)],
                    lhsT=ones_bf[HD : HD + 1, 0:HD],
                    rhs=rc[HD : HD + 1, 512 * p : 512 * (p + 1)],
                    start=True,
                    stop=True,
                )
            rbs = rbsp.tile([HD, 1024], BF16, tag="rbs", name=f"rbs{tag}_{j}")
            nc.vector.tensor_copy(rbs[:], rb[0:HD, :])
            st = stp.tile([P, 512], BF16, tag="st", name=f"st{tag}_{j}")
            st_t[j] = st
            stt = sttp.tile([HD, 512], BF16, tag="stt", name=f"stt{tag}_{j}")
            nc.vector.tensor_mul(st[0:HD, :], pv[0][0:HD, :], rbs[:, 0:512])
            nc.vector.tensor_mul(stt[:], pv[1][0:HD, :], rbs[:, 512:1024])
            # stack head1 under head0; the last block uses the idle ACT
            # queue to dodge sync-queue latency in the exposed tail
            eng = nc.scalar if j == NSB - 1 else nc.sync
            eng.dma_start(st[HD:P, :], stt[:])

        def wo_partial(j):
            # local partial wo: out[128-outs-block, 512 tokens] x4 blocks,
            # K=128 local head dims; bias added during the PSUM drain.
            st = st_t.pop(j)
            c = next(i for i in range(NRS - 1, -1, -1) if RS_FIRST[i] <= j)
            half = j - RS_FIRST[c]
            for h2 in range(2):
                wop = scp.tile([P, 1024], F32, tag="sc", name=f"wop{tag}_{j}_{h2}")
                stg = stgp.tile([P, 2, 512], BF16, tag="stg", name=f"stg{tag}_{j}_{h2}")
                for ob in range(2):
                    blk = 2 * h2 + ob
                    nc.tensor.matmul(
                        wop[:, 512 * ob : 512 * (ob + 1)],
                        lhsT=wo_sb[:, blk, :],
                        rhs=st[:],
                        start=True,
                        stop=True,
                    )
                    nc.vector.tensor_scalar_add(
                        stg[:, ob, :],
                        wop[:, 512 * ob : 512 * (ob + 1)],
                        wob_sb[:, blk : blk + 1],
                    )
                eng2 = nc.scalar if j == NSB - 1 and h2 == 1 else nc.sync
                eng2.dma_start(
                    partial[c][
                        256 * h2 : 256 * (h2 + 1), 512 * half : 512 * (half + 1)
                    ].rearrange("(b p) t -> p b t", p=P),
                    stg[:],
                )

        def reduce_scatter(c):
            if do_collective:
                nc.gpsimd.collective_compute(
                    "ReduceScatter",
                    mybir.AluOpType.add,
                    replica_groups=[[0, 1, 2, 3], [4, 5, 6, 7]],
                    ins=[partial[c][:].opt()],
                    outs=[rsout[c][:].opt()],
                )
                # out-copy: gpsimd queue mid-kernel (orders behind its RS,
                # blocks nothing); the last chunk splits across the two idle
                # HWDGE queues to halve the exposed tail copy
                if c == NRS - 1:
                    nc.sync.dma_start(out_d[c][0 : P // 2, :], rsout[c][0 : P // 2, :])
                    nc.scalar.dma_start(out_d[c][P // 2 : P, :], rsout[c][P // 2 : P, :])
                else:
                    nc.gpsimd.dma_start(out_d[c][:], rsout[c][:])
            else:
                nc.sync.dma_start(out_d[c][:], partial[c][0:P, :])

        # ---- schedule ----
        # Injection points spread sub-microsecond PE pieces across the
        # exp-paced groups so ScalarE never starves: after g0 the previous
        # block's normalization, after g1 its wo partials (+due RS), after
        # g2/g3 the next block's K/Q and V projections.
        pv_t = {}

        def make_pv(j):
            pv_t[j] = [
                pvp.tile([P, 512], F32, tag="pv", name=f"pvt{tag}_{p}_{j}")
                for p in range(2)
            ]

        xt_t = {}
        load_consts_first()
        proj_dma(0, xt_t)
        load_consts_early()
        proj_kq(0, xt_t)
        load_consts_late()
        proj_v(0, xt_t)
        make_pv(0)
        pending = []  # deferred PV: flushed one group behind QK
        for j in range(NSB):
            ng = 2 * (j + 1)
            if j + 1 < NSB:
                proj_dma(j + 1, xt_t)
            pieces = []
            if j > 0:

                def norm_piece(jj=j):
                    while pending and pending[0][0] == jj - 1:
                        attn_pv(*pending.pop(0))
                    norm(jj - 1, pv_t.pop(jj - 1))

                pieces.append(norm_piece)

                def wo_rs(jj=j):
                    wo_partial(jj - 1)
                    for c in range(NRS):
                        if RS_FIRST[c] + RS_BLOCKS[c] == jj:
                            reduce_scatter(c)

                pieces.append(wo_rs)
            if j + 1 < NSB:
                pieces.append(lambda jj=j: proj_kq(jj + 1, xt_t))
                pieces.append(lambda jj=j: (proj_v(jj + 1, xt_t), make_pv(jj + 1)))
            stride = max(2, ng // 4)
            for g in range(ng):
                pt_ = attn_qk(j, g)
                if len(pending) >= 3:
                    attn_pv(*pending.pop(0))
                pending.append((j, g, pt_, pv_t[j]))
                if pieces and (ng == 2 or (g >= 1 and (g - 1) % stride == 0)):
                    pieces.pop(0)()
            while pieces:
                pieces.pop(0)()
        while pending:
            attn_pv(*pending.pop(0))
        norm(NSB - 1, pv_t.pop(NSB - 1))
        wo_partial(NSB - 1)
        reduce_scatter(NRS - 1)


def _get_nc():
    if "nc" not in _CACHE:
        _CACHE["nc"] = _build_nc()
    return _CACHE["nc"]


def _prepare_in_maps(x, wq_w, wq_b, wk_w, wk_b, wv_w, wv_b, wo_w, wo_b):
    bf16 = ml_dtypes.bfloat16
    f32 = np.float32
    x = np.asarray(x, f32)
    wq_w = np.asarray(wq_w, f32)
    wq_b = np.asarray(wq_b, f32)
    wk_w = np.asarray(wk_w, f32)
    wk_b = np.asarray(wk_b, f32)
    wv_w = np.asarray(wv_w, f32)
    wv_b = np.asarray(wv_b, f32)
    wo_w = np.asarray(wo_w, f32)
    wo_b = np.asarray(wo_b, f32)

    scale = f32(1.0 / math.sqrt(D))

    qi = np.arange(512)[None, :]
    ki = np.arange(P)[:, None]
    masks = np.stack(
        [(ki + 128 * c <= qi).astype(f32) for c in range(4)], axis=0
    )  # [4,128,512]
    masks_bf = np.ascontiguousarray(masks.astype(bf16))

    xT = [np.ascontiguousarray(x[b].T).astype(bf16) for b in range(B)]

    in_maps = []
    for i in range(8):
        b = i // 4
        r = i % 4
        h0 = 2 * r
        hs = slice(64 * h0, 64 * h0 + 128)
        # per-core wo bias: fold wv_b through this core's wo columns; the
        # full wo_b rides on group-rank 0 only (summed once by the RS).
        wob_core = wo_w[:, hs] @ wv_b[hs]
        if r == 0:
            wob_core = wob_core + wo_b
        in_maps.append(
            {
                "xT": xT[b],
                "wqT": np.ascontiguousarray((wq_w[hs, :] * scale).T).astype(bf16),
                "wkT": np.ascontiguousarray(wk_w[hs, :].T).astype(bf16),
                "wvT": np.ascontiguousarray(wv_w[hs, :].T).astype(bf16),
                "woT": np.ascontiguousarray(wo_w[:, hs].T).astype(bf16),
                "bq": np.ascontiguousarray((wq_b[hs] * scale).reshape(P, 1)),
                "bk": np.ascontiguousarray(wk_b[hs].reshape(P, 1)),
                "wob": np.ascontiguousarray(wob_core.reshape(NKT, P).T),
                "masks": masks_bf,
            }
        )
    return in_maps


def kernel(
    x, wq_w, wq_b, wk_w, wk_b, wv_w, wv_b, wo_w, wo_b, trace=False, **run_kwargs
):
    in_maps = _prepare_in_maps(x, wq_w, wq_b, wk_w, wk_b, wv_w, wv_b, wo_w, wo_b)
    res = run_bass_kernel_spmd(
        _get_nc(), in_maps, core_ids=list(range(8)), trace=trace, **run_kwargs
    )
    _CACHE["last_result"] = res
    out = np.zeros((B, S, D), np.float32)
    for i in range(8):
        b, r = i // 4, i % 4
        for c in range(NRS):
            oT = res.results[i][f"outT{c}"]  # [128, 512*nb]
            t0 = 512 * RS_FIRST[c]
            out[b, t0 : t0 + oT.shape[1], 128 * r : 128 * (r + 1)] = oT.T
    return out
